# revision 1
# baseline (speedup 1.0000x reference)
"""Causal self-attention (single head, d=1024) on 8 Trainium2 NeuronCores.

Problem: x [4, 2048, 1024] f32, Wq/Wk/Wv [1024, 1024] f32
         out[b] = softmax(causal((x@Wq)(x@Wk)^T / 32)) @ (x@Wv)

Sharding: 8 cores = 4 batches x 2 query-shards. Per batch, the 2048
positions form 16 chunks of 128; core parity p owns global chunks {2j+p}.
The host hands each core its batch's x (and x^T) with KEY ROWS/COLUMNS
PERMUTED "mine-first": permuted block m<8 is the core's own chunk m
(global 2m+p), block m>=8 is the partner's chunk m-8 (global 2(m-8)+1-p).
The core's own 1024 query/key positions are the contiguous first half, so
one compiled SPMD program serves both parities; the only parity-dependent
bit rides in via the tiny `pval` input (masks the partner diagonal block).

Associativity tricks (the whole point of this kernel): the per-core Q and
K projections and the V projection over all 2048 keys are never formed.
Instead
    S  = Xq (Wq Wk^T) Xk^T   ->  M = Wq Wk^T, A^T = M^T Xq^T, S^T = Xk A^T
    out = softmax(S) Xk Wv   ->  Z^T = Xk^T es,  out = Z Wv / den
which turns 2048-key projections into 1024-query-sized matmuls. Per-core
PE work drops from 475k rows (Q+K+V proj + S + attv) to 344k rows
(M 64k + A 64k + S 74k + Z 74k + out 64k + den), a 1.38x reduction, with
no cross-core communication.

Per core (all matmuls bf16, fp32 PSUM; out[i,j] = sum_p lhsT[p,i] rhs[p,j]):
  M[d1, d2]  = sum_e  WqT[e, d1] WkT[e, d2]
  AT[d2, q]  = sum_d1 M[d1, d2] xT[d1, q]          (queries = cols 0:1024)
  S^T[k, q]  = sum_d2 xT[d2, k] AT[d2, q]          (causal extents)
  es         = exp(S^T / 32)  (no max-subtraction: logits ~N(0,1))
  es[kb<8][:, 0:128]  *= triangle  (own-chunk diagonal)
  es[kb>=8][:, 0:128] *= pval      (partner diag: 0.0 if p==0 else 1.0)
  den[q]     = sum_k es[k, q]   (PE, ones column)
  ZT[d, q]   = sum_k xn[k, d] es[k, q]             (causal extents)
  out[q, e]  = sum_d ZT[d, q] Wv[d, e] * (1/den[q])

Causal extents: query chunk qb needs permuted key blocks {0..qb, 8..8+qb} -
72 of 128 block pairs, 56% of the full score/Z work, balanced across cores.
"""

import sys

for _p in ("/opt/trn_rl_repo", "/root/.axon_site/_ro/trn_rl_repo"):
    if _p not in sys.path:
        sys.path.append(_p)

import numpy as np
import ml_dtypes

import concourse.bass as bass
import concourse.mybir as mybir
from concourse.tile import TileContext
from concourse import bass_utils

BF16 = mybir.dt.bfloat16
F32 = mybir.dt.float32

B, T, D = 4, 2048, 1024
NCORES = 8
P = 128
ND = D // P            # 8 tiles over d_in / d_out
NKB = T // P           # 16 key blocks
NCH = 8                # local query chunks per core
CH = 128               # chunk width
DQ = NCH * CH          # 1024 local queries per core
SCALE = 1.0 / np.sqrt(np.float32(D))  # 1/32


def _split_multiwait(nc):
    """This walrus build rejects >1-2 sync waits per instruction for several
    encodings (CTRL drains, PSEUDO_DMA...: "Too many sync wait commands").
    Tile can emit many waits on one instruction. Hoist all but the last wait
    of any multi-wait instruction onto NoOps on the same engine immediately
    before it - same-engine program order makes this equivalent."""
    for f in nc.m.functions:
        for bb in f.blocks:
            newlist = []
            changed = False
            for ins in bb.instructions:
                si = ins.sync_info
                waits = list(si.on_wait) if si and si.on_wait else []
                if len(waits) > 1:
                    changed = True
                    extra, keep = waits[:-1], waits[-1:]
                    for i, w in enumerate(extra):
                        nop = mybir.InstNoOp(
                            name=f"{ins.name}-sw{i}",
                            opcode="NoOp",
                            engine=ins.engine,
                            sync_info=mybir.SyncInfo(on_wait=[w], on_update=[]),
                        )
                        newlist.append(nop)
                    ins.sync_info = mybir.SyncInfo(
                        on_wait=keep, on_update=list(si.on_update)
                    )
                newlist.append(ins)
            if changed:
                bb.instructions = newlist


def _col_groups(qlo):
    """Split columns [qlo, DQ) into matmul groups of width <=512."""
    w = DQ - qlo
    out = []
    o = qlo
    while w > 0:
        g = min(512, w)
        out.append((o, g))
        o += g
        w -= g
    return out


def _build(split=True, reps=1):
    nc = bass.Bass("TRN2", target_bir_lowering=False, debug=False, num_devices=NCORES)

    xT = nc.declare_dram_parameter("xT", [D, T], BF16, isOutput=False)
    xn_d = nc.declare_dram_parameter("xn", [T, D], BF16, isOutput=False)
    wqT_d = nc.declare_dram_parameter("WqT", [D, D], BF16, isOutput=False)
    wkT_d = nc.declare_dram_parameter("WkT", [D, D], BF16, isOutput=False)
    wv_d = nc.declare_dram_parameter("Wv", [D, D], BF16, isOutput=False)
    tri_d = nc.declare_dram_parameter("tri", [P, CH], BF16, isOutput=False)
    pv_d = nc.declare_dram_parameter("pval", [P, 1], F32, isOutput=False)
    out = nc.declare_dram_parameter("out", [DQ, D], F32, isOutput=True)

    exp_f = mybir.ActivationFunctionType.Exp

    with (
        TileContext(nc) as tc,
        # pools OUTSIDE the rep loop: pool close inserts an all-engine
        # barrier, which would serialize reps and expose the input DMAs
        tc.tile_pool(name="pm", bufs=ND) as pm,
        tc.tile_pool(name="pat", bufs=ND) as pat,
        tc.tile_pool(name="pzt", bufs=ND) as pzt,
        tc.tile_pool(name="pconst", bufs=1) as pconst,
        tc.tile_pool(name="pes", bufs=2) as pes,
        tc.tile_pool(name="pout", bufs=2) as pout,
        tc.tile_pool(name="psm", bufs=NCH) as psm,
        tc.tile_pool(name="px", bufs=2) as px,
        tc.tile_pool(name="pw", bufs=3) as pw,
        tc.tile_pool(name="pmm", bufs=2, space="PSUM") as pmm,
        tc.tile_pool(name="pacc", bufs=4, space="PSUM") as pacc,
        tc.tile_pool(name="pden", bufs=2, space="PSUM") as pden,
    ):
      for rep in range(reps):
        if True:
            ones = pconst.tile([P, 8], BF16)
            nc.vector.memset(ones, 1.0)
            tri = pconst.tile([P, CH], BF16)
            nc.gpsimd.dma_start(out=tri, in_=tri_d[:, :])
            pval = pconst.tile([P, 1], F32)
            nc.gpsimd.dma_start(out=pval, in_=pv_d[:, :])

            # ---- input DMAs -------------------------------------------------
            # Phase order is M -> AT -> S -> den -> ZT -> out, so the
            # M operands (WqT / WkT) stream first, then xT (AT needs the
            # query half, S the rest), then xn, then Wv.
            wqT_r = wqT_d.rearrange("(e p) d -> p e d", p=P)
            wkT_r = wkT_d.rearrange("(e p) d -> p e d", p=P)
            wv_r = wv_d.rearrange("(d p) e -> p d e", p=P)
            xT_r = xT.rearrange("(d p) t -> p d t", p=P)
            xn_r = xn_d.rearrange("(k p) d -> p k d", p=P)

            wqTt = pw.tile([P, ND, D], BF16, name="wqTt", tag="wq", bufs=1)
            wkTt = pw.tile([P, ND, D], BF16, name="wkTt", tag="wk", bufs=1)
            # first M group needs WqT[:, :, 0:128] + WkT[:, :, 0:512]
            nc.sync.dma_start(out=wqTt[:, :, 0:256], in_=wqT_r[:, :, 0:256])
            nc.scalar.dma_start(out=wkTt[:, :, 0:512], in_=wkT_r[:, :, 0:512])
            nc.scalar.dma_start(out=wkTt[:, :, 512:1024], in_=wkT_r[:, :, 512:1024])
            for qq in range(1, 4):
                nc.sync.dma_start(
                    out=wqTt[:, :, 256 * qq : 256 * (qq + 1)],
                    in_=wqT_r[:, :, 256 * qq : 256 * (qq + 1)],
                )
            xt = px.tile([P, ND, T], BF16, name="xt", tag="xt", bufs=1)
            # Bulk x loads on the SP hardware queue (ACT's sequencer must
            # remain free for the PSUM->SBUF copies), paired [P,2,...] like
            # xn's chunks -- wider single DMAs fail at runtime.
            for d in range(0, ND, 2):
                nc.sync.dma_start(
                    out=xt[:, d : d + 2, 0:DQ], in_=xT_r[:, d : d + 2, 0:DQ]
                )
            for d in range(0, ND, 2):
                nc.sync.dma_start(
                    out=xt[:, d : d + 2, DQ:T], in_=xT_r[:, d : d + 2, DQ:T]
                )
            xn = px.tile([P, NKB, D], BF16, name="xn", tag="xn", bufs=1)
            for k2 in range(NKB // 2):
                nc.sync.dma_start(
                    out=xn[:, 2 * k2 : 2 * k2 + 2, :], in_=xn_r[:, 2 * k2 : 2 * k2 + 2, :]
                )
            wvt = pw.tile([P, ND, D], BF16, name="wvt", tag="wv", bufs=1)
            # scalar-queue FIFO puts this 2MB load BEHIND the two WkT DMAs,
            # keeping startup bandwidth for the M-phase operands (the
            # semaphore-gated version of this is runtime-rejected)
            nc.scalar.dma_start(out=wvt, in_=wv_r)

            # ---- Phase 1: M[d1, d2] = sum_e WqT[e, d1] WkT[e, d2] ----------
            m = []
            for d1 in range(ND):
                mt = pm.tile([P, D], BF16, name=f"m{d1}", tag="m")
                m.append(mt)
                d1s = slice(d1 * P, (d1 + 1) * P)
                for g in range(2):
                    gs = slice(g * 512, (g + 1) * 512)
                    ps = pmm.tile([P, 512], F32, name="psm_", tag="mm")
                    for e in range(ND):
                        nc.tensor.matmul(
                            ps,
                            lhsT=wqTt[:, e, d1s],
                            rhs=wkTt[:, e, gs],
                            start=(e == 0),
                            stop=(e == ND - 1),
                        )
                    nc.scalar.copy(mt[:, gs], ps)

            # ---- Phase 2: AT[d2, q] = sum_d1 M[d1, d2] xT[d1, q] -----------
            at = []
            for d2 in range(ND):
                att = pat.tile([P, DQ], BF16, name=f"at{d2}", tag="at")
                at.append(att)
                d2s = slice(d2 * P, (d2 + 1) * P)
                for g in range(2):
                    gs = slice(g * 512, (g + 1) * 512)
                    ps = pmm.tile([P, 512], F32, name="psa", tag="mm")
                    for d1 in range(ND):
                        nc.tensor.matmul(
                            ps,
                            lhsT=m[d1][:, d2s],
                            rhs=xt[:, d1, gs],
                            start=(d1 == 0),
                            stop=(d1 == ND - 1),
                        )
                    nc.scalar.copy(att[:, gs], ps)

            # ---- Phase 3: es[kb] = exp(S^T/32), S^T[k, q] = Xk A^T ---------
            es = []
            for kb in range(NKB):
                qlo = (kb % NCH) * CH
                wdt = DQ - qlo
                ksl = slice(kb * P, (kb + 1) * P)
                t_es = pes.tile([P, wdt], BF16, name=f"es{kb}", tag=f"es{wdt}")
                es.append((t_es, qlo))
                for o, g in _col_groups(qlo):
                    ps = pmm.tile([P, g], F32, name="pss", tag="mm")
                    for d2 in range(ND):
                        nc.tensor.matmul(
                            ps,
                            lhsT=xt[:, d2, ksl],
                            rhs=at[d2][:, o : o + g],
                            start=(d2 == 0),
                            stop=(d2 == ND - 1),
                        )
                    nc.scalar.activation(
                        t_es[:, o - qlo : o - qlo + g], ps, exp_f,
                        scale=float(SCALE),
                    )
                if kb < NCH:
                    nc.vector.tensor_mul(t_es[:, 0:CH], t_es[:, 0:CH], tri)
                else:
                    nc.vector.tensor_scalar_mul(t_es[:, 0:CH], t_es[:, 0:CH], pval)

            # ---- Phase 4+5: ZT[d, q] = sum_k xn[k, d] es[k, q], with the
            # den[q] chains (qb = d) interleaved between ZT's wide matmuls so
            # the 1-row den matmuls' fixed SBUF-access latency hides under
            # neighbours' streams. kb inner-ordered so each lhsT (xn block)
            # serves both q-groups.
            rds = [None] * NCH
            zt = []
            for d in range(ND):
                ztt = pzt.tile([P, DQ], BF16, name=f"zt{d}", tag="zt")
                zt.append(ztt)
                ds = slice(d * P, (d + 1) * P)
                pz = [
                    pacc.tile([P, 512], F32, name=f"pz{d}_{g}", tag="zacc")
                    for g in range(2)
                ]
                pd = pden.tile([P, 8], F32, name=f"pd{d}", tag="den")
                last = {0: 3 + NCH, 1: NCH + 7}  # last valid kb per group
                for kb in range(NKB):
                    qlo = (kb % NCH) * CH
                    t_es, _ = es[kb]
                    for g in range(2):
                        glo, ghi = 512 * g, 512 * (g + 1)
                        lo = max(qlo, glo)
                        if lo >= ghi:
                            continue
                        nc.tensor.matmul(
                            pz[g][:, lo - glo : 512],
                            lhsT=xn[:, kb, ds],
                            rhs=t_es[:, lo - qlo : ghi - qlo],
                            start=(kb == 0),
                            stop=(kb == last[g]),
                        )
                    if kb <= d or NCH <= kb <= NCH + d:
                        # den chain for query chunk qb=d: block kb's step
                        lh = t_es[:, d * P - qlo : d * P - qlo + P]
                        nc.tensor.matmul(
                            pd[:, 0:1], lhsT=lh, rhs=ones[:, 0:1],
                            start=(kb == 0), stop=(kb == NCH + d),
                        )
                rd = psm.tile([P, 1], F32, name=f"rd{d}", tag="rd")
                nc.vector.reciprocal(rd, pd[:, 0:1])
                rds[d] = rd
                for g in range(2):
                    nc.scalar.copy(ztt[:, 512 * g : 512 * (g + 1)], pz[g])

            # ---- Phase 6: out[q, e] = (sum_d ZT[d, q] Wv[d, e]) / den ------
            for qb in range(NCH):
                qs = slice(qb * P, (qb + 1) * P)
                po = [
                    pacc.tile([P, 512], F32, name=f"po{qb}_{g}", tag="zacc")
                    for g in range(2)
                ]
                for d in range(ND):
                    for g in range(2):
                        nc.tensor.matmul(
                            po[g],
                            lhsT=zt[d][:, qs],
                            rhs=wvt[:, d, 512 * g : 512 * (g + 1)],
                            start=(d == 0),
                            stop=(d == ND - 1),
                        )
                ot = pout.tile([P, D], F32, name=f"ot{qb}", tag="ot")
                # normalize + store in 256-col pieces so the final DMA
                # overlaps the preceding normalizes (shorter kernel tail)
                for piece in range(4):
                    lo, hi = piece * 256, (piece + 1) * 256
                    nc.vector.tensor_scalar_mul(
                        ot[:, lo:hi], po[piece // 2][:, lo % 512 : lo % 512 + 256],
                        rds[qb],
                    )
                    eng = nc.sync if piece % 2 == 0 else nc.scalar
                    eng.dma_start(out=out[qs, lo:hi], in_=ot[:, lo:hi])

    if split:
        _split_multiwait(nc)
    return nc


def _build_trivial():
    """Same I/O signature as _build(), near-empty body. Used by the timing
    harness to measure the per-dispatch overhead (tunnel RPC + NEFF launch),
    which is then subtracted from the real kernel's per-dispatch time."""
    nc = bass.Bass("TRN2", target_bir_lowering=False, debug=False, num_devices=NCORES)
    nc.declare_dram_parameter("xT", [D, T], BF16, isOutput=False)
    nc.declare_dram_parameter("xn", [T, D], BF16, isOutput=False)
    nc.declare_dram_parameter("WqT", [D, D], BF16, isOutput=False)
    nc.declare_dram_parameter("WkT", [D, D], BF16, isOutput=False)
    nc.declare_dram_parameter("Wv", [D, D], BF16, isOutput=False)
    tri_d = nc.declare_dram_parameter("tri", [P, CH], BF16, isOutput=False)
    nc.declare_dram_parameter("pval", [P, 1], F32, isOutput=False)
    out = nc.declare_dram_parameter("out", [DQ, D], F32, isOutput=True)
    with TileContext(nc) as tc:
        with tc.tile_pool(name="p", bufs=1) as pool:
            t = pool.tile([P, CH], BF16)
            nc.sync.dma_start(out=t, in_=tri_d[:, :])
            t2 = pool.tile([P, CH], F32)
            nc.vector.tensor_scalar_mul(t2, t, 1.0)
            nc.sync.dma_start(out=out[0:P, 0:CH], in_=t2)
    _split_multiwait(nc)
    return nc


_NC = None


def _get_nc():
    global _NC
    if _NC is None:
        _NC = _build()
    return _NC


def _perm(p):
    """Permuted key order for a parity-p core: block m<8 is own chunk m
    (global 2m+p), block m>=8 is partner chunk m-8 (global 2(m-8)+1-p)."""
    c = np.arange(T)
    m = c // CH
    off = c % CH
    chunk = np.where(m < NCH, 2 * m + p, 2 * (m - NCH) + 1 - p)
    return CH * chunk + off


def _local_to_global_q(p):
    """Map local query index [0, DQ) of a parity-p core to global [0, T)."""
    l = np.arange(DQ)
    return CH * (2 * (l // CH) + p) + (l % CH)


def _make_inputs(x, Wq, Wk, Wv):
    bf = ml_dtypes.bfloat16
    wqT = np.ascontiguousarray(Wq.T.astype(bf))
    wkT = np.ascontiguousarray(Wk.T.astype(bf))
    wvb = np.ascontiguousarray(Wv.astype(bf))

    tri = (np.arange(P)[:, None] <= np.arange(CH)[None, :]).astype(bf)
    pvals = [np.full((P, 1), float(p), np.float32) for p in range(2)]
    perms = [_perm(p) for p in range(2)]

    in_maps = []
    for c in range(NCORES):
        b, p = c // 2, c % 2
        xb = x[b].astype(bf)            # [T, D]
        xnp = np.ascontiguousarray(xb[perms[p], :])
        xTp = np.ascontiguousarray(xnp.T)
        in_maps.append(
            {"xT": xTp, "xn": xnp, "WqT": wqT, "WkT": wkT, "Wv": wvb,
             "tri": tri, "pval": pvals[p]}
        )
    return in_maps


def _assemble(results, dtype=np.float32):
    y = np.empty((B, T, D), dtype=dtype)
    for c in range(NCORES):
        b, p = c // 2, c % 2
        y[b, _local_to_global_q(p), :] = results[c]["out"]
    return y


def run_spmd(x, Wq, Wk, Wv, **kwargs):
    """Run the kernel; returns (full_output, BassKernelResults)."""
    nc = _get_nc()
    in_maps = _make_inputs(
        np.asarray(x, np.float32),
        np.asarray(Wq, np.float32),
        np.asarray(Wk, np.float32),
        np.asarray(Wv, np.float32),
    )
    r = bass_utils.run_bass_kernel_spmd(nc, in_maps, core_ids=list(range(NCORES)), **kwargs)
    return _assemble(r.results), r


def kernel(x, Wq, Wk, Wv):
    y, _ = run_spmd(x, Wq, Wk, Wv)
    return y

